# revision 1
# baseline (speedup 1.0000x reference)
"""CSPN (7x7 per-pixel spatial propagation) Trainium2 kernel.

Problem: out[b,0,y,x] = sum_{i,j in 0..6} gw[b, 7i+j, y+3, x+3] * src(y+3-i, x+3-j)
where src = hn (zero-padded outside [0,512)) except the center tap (i=j=3)
which uses h0. Shapes: gw [8,49,518,518] f32, hn/h0 [8,1,512,512] f32.

Strategy: pure data parallel - one batch element per NeuronCore (8 cores).
Per core the 512x512 image lives as [128 partitions, 4 row-blocks, 512
cols]. The guide-weight read window is identical for every tap (rows/cols
3:515), so each tap is one ~1MB DMA; that 51.4MB/core stream is the
memory-roofline term.

Engine/queue roles (chosen to avoid sequencer convoys - a HWDGE dma_start
waiting on a semaphore blocks every later instruction on that sequencer):
 - Sync + GpSimd sequencers: pure DMA issue rings for the weight stream
   (round-robin), so the 16 SDMA engines always have a second descriptor
   ring to drain during one ring's completion gap.
 - Scalar engine: only the f32->bf16 weight casts (so DVE multiplies run
   in 2x mode) plus the final output stores.
 - Vector engine: per-tap multiply + 49-term bf16 accumulation, halo
   plane casts.

The per-tap source shift is absorbed by a zero-padded bf16 halo tensor
s0[p, k, b, u] = hn[128b+p+k-3, u-3]; a second copy s1 one u-slot later
keeps bf16 reads 4B-aligned for odd-j taps. Each partition-shifted plane
is staged in f32 by SBUF->SBUF DMA from the raw hn staging tile (no cast
dependency - engine ops cannot partition-shift, DMAs can) on the GpSimd
ring, then cast to s0/s1 on the DVE. Planes build one image-row ahead of
the tap chain. The last three taps run block-striped (per-row-block
weight quarters, accumulate, cast, store) so the tail drains as a
pipeline behind the final weight bytes.
"""

import numpy as np

_CACHE = {}


def _build_nc():
    import concourse.bacc as bacc
    import concourse.mybir as mybir
    import concourse.tile as tile

    F32 = mybir.dt.float32
    BF16 = mybir.dt.bfloat16
    MULT = mybir.AluOpType.mult
    ADD = mybir.AluOpType.add

    nc = bacc.Bacc("TRN2", target_bir_lowering=False, debug=False, num_devices=8)
    gw = nc.dram_tensor("gw", [49, 518, 518], F32, kind="ExternalInput").ap()
    hn = nc.dram_tensor("hn", [512, 512], F32, kind="ExternalInput").ap()
    h0 = nc.dram_tensor("h0", [512, 512], F32, kind="ExternalInput").ap()
    out = nc.dram_tensor("out", [512, 512], F32, kind="ExternalOutput").ap()

    with tile.TileContext(nc) as tc:
        with (
            tc.tile_pool(name="persist", bufs=1) as pp,
            tc.tile_pool(name="wf", bufs=7) as wfp,
            tc.tile_pool(name="ftmp", bufs=2) as ftp,
            tc.tile_pool(name="wb", bufs=5) as wbp,
            tc.tile_pool(name="prod", bufs=3) as prp,
            tc.tile_pool(name="wtail", bufs=5) as wtp,
        ):
            # Stage hn/h0 as [p, b, x]; h0's bf16 cast runs on Scalar.
            hn_r = hn.rearrange("(b p) x -> p b x", p=128)
            hnf = pp.tile([128, 4, 512], F32, tag="stage_a")
            nc.sync.dma_start(out=hnf[:], in_=hn_r)
            h0f = pp.tile([128, 4, 512], F32)
            nc.sync.dma_start(out=h0f[:], in_=h0.rearrange("(b p) x -> p b x", p=128))
            h0b = pp.tile([128, 4, 512], BF16)
            nc.scalar.copy(out=h0b[:], in_=h0f[:])

            # Halo tensors: s0[p, k, b, u] = hn[128b+p+k-3, u-3] (zero outside
            # the image), s1 the same data one u-slot later so odd-j taps read
            # 4B-aligned.
            s0 = pp.tile([128, 7, 4, 520], BF16, tag="s0")
            s1 = pp.tile([128, 7, 4, 520], BF16, tag="s1")
            nc.vector.memset(s0[:, :, :, 0:3], 0.0)
            nc.vector.memset(s0[:, :, :, 515:520], 0.0)
            nc.vector.memset(s1[:, :, :, 0:4], 0.0)
            nc.vector.memset(s1[:, :, :, 516:520], 0.0)
            # Zero strip used to clear halo staging gap rows via DMA (DMAs
            # have no partition-alignment constraints, engine memsets do).
            zt = pp.tile([32, 512], F32, tag="zt")
            nc.vector.memset(zt[:], 0.0)

            def build_plane(k):
                d = k - 3
                if d == 0:
                    # Unshifted plane: cast straight from hnf on the DVE.
                    nc.vector.tensor_copy(s0[:, 3, :, 3:515], hnf[:])
                    nc.vector.tensor_copy(s1[:, 3, :, 4:516], hnf[:])
                    return
                # Partition-shifted plane staged in f32 straight from DRAM hn
                # (no dependencies, normal HBM->SBUF direction - SBUF->SBUF
                # staging starves against the weight stream's port traffic),
                # then cast to both bf16 copies on the DVE. Gap rows hold
                # garbage in the staging tile; they are re-zeroed in s0/s1
                # right after the casts (same DVE FIFO, no ring stalls).
                ft = ftp.tile([128, 4, 512], F32, tag="ftmp")
                eng = nc.sync if k % 2 == 0 else nc.scalar
                if d > 0:
                    eng.dma_start(out=ft[0 : 128 - d, 0:4, :], in_=hn_r[d:128, 0:4, :])
                    eng.dma_start(out=ft[128 - d : 128, 0:3, :], in_=hn_r[0:d, 1:4, :])
                    eng.dma_start(out=ft[128 - d : 128, 3, :], in_=zt[0:d, :])
                else:
                    eng.dma_start(out=ft[-d:128, 0:4, :], in_=hn_r[0 : 128 + d, 0:4, :])
                    eng.dma_start(out=ft[0:-d, 1:4, :], in_=hn_r[128 + d : 128, 0:3, :])
                    eng.dma_start(out=ft[0:-d, 0, :], in_=zt[0:-d, :])
                nc.vector.tensor_copy(s0[:, k, :, 3:515], ft[:])
                nc.vector.tensor_copy(s1[:, k, :, 4:516], ft[:])

            acc = pp.tile([128, 4, 512], BF16)
            outf = pp.tile([128, 4, 512], F32, tag="stage_a")
            out_ap = out.rearrange("(b p) x -> p b x", p=128)

            def src_for(t):
                i, j = t // 7, t % 7
                if t == 24:
                    return h0b[:]
                if j % 2 == 0:
                    return s0[:, 6 - i, :, 6 - j : 518 - j]
                return s1[:, 6 - i, :, 7 - j : 519 - j]

            # Taps 0..45 stream full-tile on the two pure-DMA rings; plane
            # k=6-i is built one image-row ahead of the taps that read it.
            build_plane(6)
            for t in range(46):
                i, j = t // 7, t % 7
                if j == 0 and i < 6:
                    build_plane(5 - i)
                wf = wfp.tile([128, 4, 512], F32, tag="wf")
                eng = nc.sync if t % 2 == 0 else nc.scalar
                eng.dma_start(
                    out=wf[:],
                    in_=gw[t, 3:515, 3:515].rearrange("(b p) x -> p b x", p=128),
                )
                # bf16 weight cast on the Scalar engine (2x DVE multiply).
                wb = wbp.tile([128, 4, 512], BF16, tag="wb")
                nc.scalar.copy(out=wb[:], in_=wf[:])
                if t == 0:
                    nc.vector.tensor_tensor(
                        out=acc[:], in0=wb[:], in1=src_for(t), op=MULT
                    )
                else:
                    prod = prp.tile([128, 4, 512], BF16, tag="prod")
                    nc.vector.tensor_tensor(
                        out=prod[:], in0=wb[:], in1=src_for(t), op=MULT
                    )
                    nc.vector.tensor_tensor(
                        out=acc[:], in0=acc[:], in1=prod[:], op=ADD
                    )

            # Tail: last three taps run block-striped (block 0's weight
            # quarters first) so each block's accumulate/cast/store drains
            # while later blocks' weights are still arriving.
            for b in range(4):
                for t in (46, 47, 48):
                    wq = wtp.tile([128, 512], F32, tag="wq")
                    eng = nc.sync if t % 2 == 0 else nc.scalar
                    eng.dma_start(
                        out=wq[:], in_=gw[t, 3 + 128 * b : 131 + 128 * b, 3:515]
                    )
                    wbq = wtp.tile([128, 512], BF16, tag="wbq")
                    nc.scalar.copy(out=wbq[:], in_=wq[:])
                    prod = prp.tile([128, 512], BF16, tag="prodb")
                    nc.vector.tensor_tensor(
                        out=prod[:], in0=wbq[:], in1=src_for(t)[:, b, :], op=MULT
                    )
                    nc.vector.tensor_tensor(
                        out=acc[:, b, :], in0=acc[:, b, :], in1=prod[:], op=ADD
                    )
                nc.scalar.copy(out=outf[:, b, :], in_=acc[:, b, :])
                nc.scalar.dma_start(out=out_ap[:, b, :], in_=outf[:, b, :])

    nc.compile()
    return nc


def get_nc():
    if "nc" not in _CACHE:
        _CACHE["nc"] = _build_nc()
    return _CACHE["nc"]


def kernel(guide_weight, hn, h0):
    from concourse.bass_utils import run_bass_kernel_spmd

    nc = get_nc()
    in_maps = [
        {
            "gw": np.ascontiguousarray(guide_weight[b], dtype=np.float32),
            "hn": np.ascontiguousarray(hn[b, 0], dtype=np.float32),
            "h0": np.ascontiguousarray(h0[b, 0], dtype=np.float32),
        }
        for b in range(8)
    ]
    res = run_bass_kernel_spmd(nc, in_maps, core_ids=list(range(8)))
    return np.stack([res.results[b]["out"] for b in range(8)])[:, None].astype(
        np.float32
    )



# revision 3
# speedup vs baseline: 1.1038x; 1.1038x over previous
"""CSPN (7x7 per-pixel spatial propagation) Trainium2 kernel.

Problem: out[b,0,y,x] = sum_{i,j in 0..6} gw[b, 7i+j, y+3, x+3] * src(y+3-i, x+3-j)
where src = hn (zero-padded outside [0,512)) except the center tap (i=j=3)
which uses h0. Shapes: gw [8,49,518,518] f32, hn/h0 [8,1,512,512] f32.

Strategy: pure data parallel - one batch element per NeuronCore (8 cores).

Layout: image row r lives at partition r//4, row-block b = r%4 (so each
partition holds 4 consecutive rows). A vertical shift of up to +-3 rows
then moves at most +-1 partition, and every shifted read window is a
plain strided slice of ONE halo tensor:

    s0[p, m, 3+c] = hn[4p + m - 3, c]   for m in [0, 10)

Tap (i, j) with dr = 3-i, dc = 3-j reads s0[:, dr+3 : dr+7, 3+dc : 515+dc]
- a [128, 4, 512] slice whose element offset is even exactly when j is
even. A second copy s1 one slot to the right serves odd-j taps so bf16
DVE reads stay 4B-aligned (2x mode). The halo tensor is built from just
three DMA loads of hn (rows 4p..4p+3 / 4p+4..4p+6 / 4p-3..4p-1) into an
f32 staging tile, one ACT cast (-> s0) and one 4x-mode DVE copy (-> s1);
no per-tap plane rebuilds exist, so after startup the DMA engines stream
nothing but the 49 guide-weight planes (51.4 MB/core, the memory-roofline
term) plus the final 1 MB output store.

Engine roles:
 - Sync (SP) sequencer: pure DMA issue ring (halo staging, h0, all weight
   planes and tail weight quarters) - no compute waits can stall it.
 - Scalar (ACT): f32->bf16 weight casts (so DVE multiplies run in 2x
   mode), halo/h0 casts, output casts + output store issue.
 - Vector (DVE): per-tap multiply + 49-term bf16 accumulation.

Tail: the last three taps run block-striped (per-row-block weight
quarters, accumulate, cast, store) so the output store drains as a
pipeline while the final weight bytes are still arriving.
"""

import numpy as np

_CACHE = {}


def _build_nc():
    import concourse.bacc as bacc
    import concourse.mybir as mybir
    import concourse.tile as tile

    F32 = mybir.dt.float32
    BF16 = mybir.dt.bfloat16
    MULT = mybir.AluOpType.mult
    ADD = mybir.AluOpType.add

    nc = bacc.Bacc("TRN2", target_bir_lowering=False, debug=False, num_devices=8)
    gw = nc.dram_tensor("gw", [49, 518, 518], F32, kind="ExternalInput").ap()
    hn = nc.dram_tensor("hn", [512, 512], F32, kind="ExternalInput").ap()
    h0 = nc.dram_tensor("h0", [512, 512], F32, kind="ExternalInput").ap()
    out = nc.dram_tensor("out", [512, 512], F32, kind="ExternalOutput").ap()

    with tile.TileContext(nc) as tc:
        with (
            tc.tile_pool(name="persist", bufs=1) as pp,
            tc.tile_pool(name="wf", bufs=8) as wfp,
            tc.tile_pool(name="wb", bufs=4) as wbp,
            tc.tile_pool(name="prod", bufs=3) as prp,
            tc.tile_pool(name="wtail", bufs=4) as wtp,
        ):
            # f32 halo staging: stage[p, m, c] = hn[4p + m - 3, c].
            stage = pp.tile([128, 10, 512], F32, tag="stage")
            # Rows outside [0, 512) must read as zero. Engine memsets cannot
            # address a 1-partition range at an arbitrary offset (BIR
            # partition-alignment rule), so zero a strip and DMA it over.
            zt = pp.tile([32, 512], F32, tag="zt")
            nc.vector.memset(zt[:], 0.0)
            nc.sync.dma_start(out=stage[0:1, 0:3, :], in_=zt[0:3, :])
            nc.sync.dma_start(out=stage[127:128, 7:10, :], in_=zt[0:3, :])
            hn_r = hn.rearrange("(p b) x -> p b x", b=4)
            nc.sync.dma_start(out=stage[:, 3:7, :], in_=hn_r)
            nc.sync.dma_start(
                out=stage[0:127, 7:10, :],
                in_=hn[4:512].rearrange("(p b) x -> p b x", b=4)[:, 0:3, :],
            )
            nc.sync.dma_start(
                out=stage[1:128, 0:3, :],
                in_=hn[1:509].rearrange("(p b) x -> p b x", b=4)[:, 0:3, :],
            )
            h0f = pp.tile([128, 4, 512], F32)
            nc.sync.dma_start(out=h0f[:], in_=h0.rearrange("(p b) x -> p b x", b=4))

            # bf16 halo tensors; s1 is s0 shifted one slot right so odd-j
            # taps read 4B-aligned.
            s0 = pp.tile([128, 10, 520], BF16, tag="s0")
            s1 = pp.tile([128, 10, 520], BF16, tag="s1")
            nc.vector.memset(s0[:, :, 0:3], 0.0)
            nc.vector.memset(s0[:, :, 515:520], 0.0)
            nc.vector.memset(s1[:, :, 0:4], 0.0)
            nc.vector.memset(s1[:, :, 516:520], 0.0)
            nc.scalar.copy(out=s0[:, :, 3:515], in_=stage[:])
            nc.vector.tensor_copy(s1[:, :, 4:516], s0[:, :, 3:515])
            h0b = pp.tile([128, 4, 512], BF16)
            nc.scalar.copy(out=h0b[:], in_=h0f[:])

            acc = pp.tile([128, 4, 512], BF16)
            outf = pp.tile([128, 4, 512], F32)
            out_r = out.rearrange("(p b) x -> p b x", b=4)
            gw_r = [
                gw[t, 3:515, 3:515].rearrange("(p b) x -> p b x", b=4)
                for t in range(49)
            ]

            def src_for(t):
                i, j = t // 7, t % 7
                if t == 24:
                    return h0b[:]
                dr, dc = 3 - i, 3 - j
                if j % 2 == 0:
                    return s0[:, dr + 3 : dr + 7, 3 + dc : 515 + dc]
                return s1[:, dr + 3 : dr + 7, 4 + dc : 516 + dc]

            # Taps 0..45 stream full-tile; weight DMAs all on the SP ring.
            for t in range(46):
                wf = wfp.tile([128, 4, 512], F32, tag="wf")
                nc.sync.dma_start(out=wf[:], in_=gw_r[t])
                wb = wbp.tile([128, 4, 512], BF16, tag="wb")
                nc.scalar.copy(out=wb[:], in_=wf[:])
                if t == 0:
                    nc.vector.tensor_tensor(
                        out=acc[:], in0=wb[:], in1=src_for(t), op=MULT
                    )
                else:
                    prod = prp.tile([128, 4, 512], BF16, tag="prod")
                    nc.vector.tensor_tensor(
                        out=prod[:], in0=wb[:], in1=src_for(t), op=MULT
                    )
                    nc.vector.tensor_tensor(
                        out=acc[:], in0=acc[:], in1=prod[:], op=ADD
                    )

            # Tail: last three taps block-striped so each row-block's
            # accumulate/cast/store drains behind the final weight bytes.
            for b in range(4):
                for t in (46, 47, 48):
                    wq = wtp.tile([128, 512], F32, tag="wq")
                    nc.sync.dma_start(out=wq[:], in_=gw_r[t][:, b, :])
                    wbq = wtp.tile([128, 512], BF16, tag="wbq")
                    nc.scalar.copy(out=wbq[:], in_=wq[:])
                    prodq = prp.tile([128, 512], BF16, tag="prodq")
                    nc.vector.tensor_tensor(
                        out=prodq[:], in0=wbq[:], in1=src_for(t)[:, b, :], op=MULT
                    )
                    nc.vector.tensor_tensor(
                        out=acc[:, b, :], in0=acc[:, b, :], in1=prodq[:], op=ADD
                    )
                nc.scalar.copy(out=outf[:, b, :], in_=acc[:, b, :])
                nc.scalar.dma_start(out=out_r[:, b, :], in_=outf[:, b, :])

    nc.compile()
    return nc


def get_nc():
    if "nc" not in _CACHE:
        _CACHE["nc"] = _build_nc()
    return _CACHE["nc"]


def kernel(guide_weight, hn, h0):
    from concourse.bass_utils import run_bass_kernel_spmd

    nc = get_nc()
    in_maps = [
        {
            "gw": np.ascontiguousarray(guide_weight[b], dtype=np.float32),
            "hn": np.ascontiguousarray(hn[b, 0], dtype=np.float32),
            "h0": np.ascontiguousarray(h0[b, 0], dtype=np.float32),
        }
        for b in range(8)
    ]
    res = run_bass_kernel_spmd(nc, in_maps, core_ids=list(range(8)))
    return np.stack([res.results[b]["out"] for b in range(8)])[:, None].astype(
        np.float32
    )


# revision 6
# speedup vs baseline: 1.1195x; 1.0142x over previous
"""CSPN (7x7 per-pixel spatial propagation) Trainium2 kernel.

Problem: out[b,0,y,x] = sum_{i,j in 0..6} gw[b, 7i+j, y+3, x+3] * src(y+3-i, x+3-j)
where src = hn (zero-padded outside [0,512)) except the center tap (i=j=3)
which uses h0. Shapes: gw [8,49,518,518] f32, hn/h0 [8,1,512,512] f32.

Strategy: pure data parallel - one batch element per NeuronCore (8 cores).

Layout: image row r lives at partition r//4, row-block b = r%4 (so each
partition holds 4 consecutive rows). A vertical shift of up to +-3 rows
then moves at most +-1 partition, and every shifted read window is a
plain strided slice of ONE halo tensor:

    s0[p, m, 3+c] = hn[4p + m - 3, c]   for m in [0, 10)

Tap (i, j) with dr = 3-i, dc = 3-j reads s0[:, dr+3 : dr+7, 3+dc : 515+dc]
- a [128, 4, 512] slice whose element offset is even exactly when j is
even. A second copy s1 one slot to the right serves odd-j taps so bf16
DVE reads stay 4B-aligned (2x mode). The halo tensor is built from just
three DMA loads of hn (rows 4p..4p+3 / 4p+4..4p+6 / 4p-3..4p-1) into an
f32 staging tile, one ACT cast (-> s0) and one 4x-mode DVE copy (-> s1);
no per-tap plane rebuilds exist, so after startup the DMA engines stream
nothing but the 49 guide-weight planes (51.4 MB/core, the memory-roofline
term) plus the final 1 MB output store.

Engine roles:
 - Sync (SP) sequencer: pure DMA issue ring (halo staging, h0, all weight
   planes and tail weight quarters) - no compute waits can stall it.
 - Scalar (ACT): f32->bf16 weight casts (so DVE multiplies run in 2x
   mode), halo/h0 casts, output casts + output store issue.
 - Vector (DVE): per-tap multiply + 49-term bf16 accumulation.

Tail: the last three taps run block-striped (per-row-block weight
quarters, accumulate, cast, store) so the output store drains as a
pipeline while the final weight bytes are still arriving.
"""

import numpy as np

_CACHE = {}


def _build_nc():
    import concourse.bacc as bacc
    import concourse.mybir as mybir
    import concourse.tile as tile

    F32 = mybir.dt.float32
    BF16 = mybir.dt.bfloat16
    MULT = mybir.AluOpType.mult
    ADD = mybir.AluOpType.add

    nc = bacc.Bacc("TRN2", target_bir_lowering=False, debug=False, num_devices=8)
    gw = nc.dram_tensor("gw", [49, 518, 518], F32, kind="ExternalInput").ap()
    hn = nc.dram_tensor("hn", [512, 512], F32, kind="ExternalInput").ap()
    h0 = nc.dram_tensor("h0", [512, 512], F32, kind="ExternalInput").ap()
    out = nc.dram_tensor("out", [512, 512], F32, kind="ExternalOutput").ap()

    with tile.TileContext(nc) as tc:
        with (
            tc.tile_pool(name="persist", bufs=1) as pp,
            tc.tile_pool(name="wf", bufs=8) as wfp,
            tc.tile_pool(name="wb", bufs=4) as wbp,
            tc.tile_pool(name="prod", bufs=3) as prp,
            tc.tile_pool(name="wtail", bufs=12) as wtp,
        ):
            # f32 halo staging: stage[p, m, c] = hn[4p + m - 3, c].
            stage = pp.tile([128, 10, 512], F32, tag="stage")
            hn_r = hn.rearrange("(p b) x -> p b x", b=4)
            nc.sync.dma_start(out=stage[:, 3:7, :], in_=hn_r)
            nc.sync.dma_start(
                out=stage[0:127, 7:10, :],
                in_=hn[4:512].rearrange("(p b) x -> p b x", b=4)[:, 0:3, :],
            )
            nc.sync.dma_start(
                out=stage[1:128, 0:3, :],
                in_=hn[1:509].rearrange("(p b) x -> p b x", b=4)[:, 0:3, :],
            )
            h0f = pp.tile([128, 4, 512], F32)
            nc.sync.dma_start(out=h0f[:], in_=h0.rearrange("(p b) x -> p b x", b=4))
            # Rows outside [0, 512) must read as zero. Engine memsets cannot
            # address a 1-partition range at an arbitrary offset (BIR
            # partition-alignment rule), so zero a strip and DMA it over;
            # issued on the otherwise-idle GpSimd ring so the SP ring's first
            # weight loads are never queued behind the memset's semaphore.
            zt = pp.tile([32, 512], F32, tag="zt")
            nc.vector.memset(zt[:], 0.0)
            nc.gpsimd.dma_start(out=stage[0:1, 0:3, :], in_=zt[0:3, :])
            nc.gpsimd.dma_start(out=stage[127:128, 7:10, :], in_=zt[0:3, :])

            # bf16 halo tensors; s1 is s0 shifted one slot right so odd-j
            # taps read 4B-aligned.
            s0 = pp.tile([128, 10, 520], BF16, tag="s0")
            s1 = pp.tile([128, 10, 520], BF16, tag="s1")
            nc.vector.memset(s0[:, :, 0:3], 0.0)
            nc.vector.memset(s0[:, :, 515:520], 0.0)
            nc.vector.memset(s1[:, :, 0:4], 0.0)
            nc.vector.memset(s1[:, :, 516:520], 0.0)
            nc.scalar.copy(out=s0[:, :, 3:515], in_=stage[:])
            nc.vector.tensor_copy(s1[:, :, 4:516], s0[:, :, 3:515])
            h0b = pp.tile([128, 4, 512], BF16)
            nc.scalar.copy(out=h0b[:], in_=h0f[:])

            acc = pp.tile([128, 4, 512], BF16)
            outf = pp.tile([128, 4, 512], F32)
            out_r = out.rearrange("(p b) x -> p b x", b=4)
            gw_r = [
                gw[t, 3:515, 3:515].rearrange("(p b) x -> p b x", b=4)
                for t in range(49)
            ]

            def src_for(t):
                i, j = t // 7, t % 7
                if t == 24:
                    return h0b[:]
                dr, dc = 3 - i, 3 - j
                if j % 2 == 0:
                    return s0[:, dr + 3 : dr + 7, 3 + dc : 515 + dc]
                return s1[:, dr + 3 : dr + 7, 4 + dc : 516 + dc]

            # Taps 0..45 stream full-tile; weight DMAs all on the SP ring.
            for t in range(46):
                wf = wfp.tile([128, 4, 512], F32, tag="wf")
                nc.sync.dma_start(out=wf[:], in_=gw_r[t])
                wb = wbp.tile([128, 4, 512], BF16, tag="wb")
                nc.scalar.copy(out=wb[:], in_=wf[:])
                if t == 0:
                    nc.vector.tensor_tensor(
                        out=acc[:], in0=wb[:], in1=src_for(t), op=MULT
                    )
                else:
                    prod = prp.tile([128, 4, 512], BF16, tag="prod")
                    nc.vector.tensor_tensor(
                        out=prod[:], in0=wb[:], in1=src_for(t), op=MULT
                    )
                    nc.vector.tensor_tensor(
                        out=acc[:], in0=acc[:], in1=prod[:], op=ADD
                    )

            # Tail: last three taps block-striped so each row-block's
            # accumulate/store drains behind the final weight bytes. All 12
            # weight quarters are buffered (bufs=12) so the SP ring streams
            # them back-to-back with no pool-reuse waits; each block's final
            # add emits f32 directly into outf (no separate ACT cast), and
            # the four stores issue from SP after the quarters.
            for b in range(4):
                prods = []
                for t in (46, 47, 48):
                    wq = wtp.tile([128, 512], F32, tag="wq")
                    nc.sync.dma_start(out=wq[:], in_=gw_r[t][:, b, :])
                    wbq = wtp.tile([128, 512], BF16, tag="wbq")
                    nc.scalar.copy(out=wbq[:], in_=wq[:])
                    prodq = wtp.tile([128, 512], BF16, tag="prodq")
                    nc.vector.tensor_tensor(
                        out=prodq[:], in0=wbq[:], in1=src_for(t)[:, b, :], op=MULT
                    )
                    prods.append(prodq)
                nc.vector.tensor_tensor(
                    out=acc[:, b, :], in0=acc[:, b, :], in1=prods[0][:], op=ADD
                )
                nc.vector.tensor_tensor(
                    out=acc[:, b, :], in0=acc[:, b, :], in1=prods[1][:], op=ADD
                )
                nc.vector.tensor_tensor(
                    out=outf[:, b, :], in0=acc[:, b, :], in1=prods[2][:], op=ADD
                )
            for b in range(4):
                nc.sync.dma_start(out=out_r[:, b, :], in_=outf[:, b, :])

    nc.compile()
    return nc


def get_nc():
    if "nc" not in _CACHE:
        _CACHE["nc"] = _build_nc()
    return _CACHE["nc"]


def kernel(guide_weight, hn, h0):
    from concourse.bass_utils import run_bass_kernel_spmd

    nc = get_nc()
    in_maps = [
        {
            "gw": np.ascontiguousarray(guide_weight[b], dtype=np.float32),
            "hn": np.ascontiguousarray(hn[b, 0], dtype=np.float32),
            "h0": np.ascontiguousarray(h0[b, 0], dtype=np.float32),
        }
        for b in range(8)
    ]
    res = run_bass_kernel_spmd(nc, in_maps, core_ids=list(range(8)))
    return np.stack([res.results[b]["out"] for b in range(8)])[:, None].astype(
        np.float32
    )


# revision 8
# speedup vs baseline: 1.1195x; 1.0000x over previous
"""CSPN (7x7 per-pixel spatial propagation) Trainium2 kernel.

Problem: out[b,0,y,x] = sum_{i,j in 0..6} gw[b, 7i+j, y+3, x+3] * src(y+3-i, x+3-j)
where src = hn (zero-padded outside [0,512)) except the center tap (i=j=3)
which uses h0. Shapes: gw [8,49,518,518] f32, hn/h0 [8,1,512,512] f32.

Strategy: pure data parallel - one batch element per NeuronCore (8 cores).

Layout: image row r lives at partition r//4, row-block b = r%4 (so each
partition holds 4 consecutive rows). A vertical shift of up to +-3 rows
then moves at most +-1 partition, and every shifted read window is a
plain strided slice of ONE halo tensor:

    s0[p, m, 3+c] = hn[4p + m - 3, c]   for m in [0, 10)

Tap (i, j) with dr = 3-i, dc = 3-j reads s0[:, dr+3 : dr+7, 3+dc : 515+dc]
- a [128, 4, 512] slice whose element offset is even exactly when j is
even. A second copy s1 one slot to the right serves odd-j taps so bf16
DVE reads stay 4B-aligned (2x mode). The halo tensor is built from just
three DMA loads of hn (rows 4p..4p+3 / 4p+4..4p+6 / 4p-3..4p-1) into an
f32 staging tile, one ACT cast (-> s0) and one 4x-mode DVE copy (-> s1);
no per-tap plane rebuilds exist, so after startup the DMA engines stream
nothing but the 49 guide-weight planes (51.4 MB/core, the memory-roofline
term) plus the final 1 MB output store.

Engine roles:
 - Sync (SP) sequencer: pure DMA issue ring (halo staging, h0, all weight
   planes and tail weight quarters) - no compute waits can stall it.
 - Scalar (ACT): f32->bf16 weight casts (so DVE multiplies run in 2x
   mode), halo/h0 casts, output casts + output store issue.
 - Vector (DVE): per-tap multiply + 49-term bf16 accumulation.

Tail: the last three taps run block-striped (per-row-block weight
quarters, accumulate, cast, store) so the output store drains as a
pipeline while the final weight bytes are still arriving.
"""

import numpy as np

_CACHE = {}


def _build_nc():
    import concourse.bacc as bacc
    import concourse.mybir as mybir
    import concourse.tile as tile

    F32 = mybir.dt.float32
    BF16 = mybir.dt.bfloat16
    MULT = mybir.AluOpType.mult
    ADD = mybir.AluOpType.add

    nc = bacc.Bacc("TRN2", target_bir_lowering=False, debug=False, num_devices=8)
    gw = nc.dram_tensor("gw", [49, 518, 518], F32, kind="ExternalInput").ap()
    hn = nc.dram_tensor("hn", [512, 512], F32, kind="ExternalInput").ap()
    h0 = nc.dram_tensor("h0", [512, 512], F32, kind="ExternalInput").ap()
    out = nc.dram_tensor("out", [512, 512], F32, kind="ExternalOutput").ap()

    with tile.TileContext(nc) as tc:
        with (
            tc.tile_pool(name="persist", bufs=1) as pp,
            tc.tile_pool(name="wf", bufs=5) as wfp,
            tc.tile_pool(name="wb", bufs=4) as wbp,
            tc.tile_pool(name="prod", bufs=2) as prp,
            tc.tile_pool(name="wq12", bufs=12) as wqp,
            tc.tile_pool(name="whalf", bufs=4) as whp,
            tc.tile_pool(name="wsmall", bufs=4) as wsp,
            tc.tile_pool(name="prodq", bufs=6) as pqp,
        ):
            # f32 halo staging: stage[p, m, c] = hn[4p + m - 3, c].
            stage = pp.tile([128, 10, 512], F32, tag="stage")
            hn_r = hn.rearrange("(p b) x -> p b x", b=4)
            nc.sync.dma_start(out=stage[:, 3:7, :], in_=hn_r)
            nc.sync.dma_start(
                out=stage[0:127, 7:10, :],
                in_=hn[4:512].rearrange("(p b) x -> p b x", b=4)[:, 0:3, :],
            )
            nc.sync.dma_start(
                out=stage[1:128, 0:3, :],
                in_=hn[1:509].rearrange("(p b) x -> p b x", b=4)[:, 0:3, :],
            )
            h0f = pp.tile([128, 4, 512], F32)
            nc.sync.dma_start(out=h0f[:], in_=h0.rearrange("(p b) x -> p b x", b=4))
            # Rows outside [0, 512) must read as zero. Engine memsets cannot
            # address a 1-partition range at an arbitrary offset (BIR
            # partition-alignment rule), so zero a strip and DMA it over;
            # issued on the otherwise-idle GpSimd ring so the SP ring's first
            # weight loads are never queued behind the memset's semaphore.
            zt = pp.tile([32, 512], F32, tag="zt")
            nc.vector.memset(zt[:], 0.0)
            nc.gpsimd.dma_start(out=stage[0:1, 0:3, :], in_=zt[0:3, :])
            nc.gpsimd.dma_start(out=stage[127:128, 7:10, :], in_=zt[0:3, :])

            # bf16 halo tensors; s1 is s0 shifted one slot right so odd-j
            # taps read 4B-aligned.
            s0 = pp.tile([128, 10, 520], BF16, tag="s0")
            s1 = pp.tile([128, 10, 520], BF16, tag="s1")
            nc.vector.memset(s0[:, :, 0:3], 0.0)
            nc.vector.memset(s0[:, :, 515:520], 0.0)
            nc.vector.memset(s1[:, :, 0:4], 0.0)
            nc.vector.memset(s1[:, :, 516:520], 0.0)
            nc.scalar.copy(out=s0[:, :, 3:515], in_=stage[:])
            nc.vector.tensor_copy(s1[:, :, 4:516], s0[:, :, 3:515])
            h0b = pp.tile([128, 4, 512], BF16)
            nc.scalar.copy(out=h0b[:], in_=h0f[:])

            acc = pp.tile([128, 4, 512], BF16)
            outf = pp.tile([128, 4, 512], F32)
            out_r = out.rearrange("(p b) x -> p b x", b=4)
            gw_r = [
                gw[t, 3:515, 3:515].rearrange("(p b) x -> p b x", b=4)
                for t in range(49)
            ]

            def src_for(t, b0=0, nb=4):
                # Source slice covering row-blocks [b0, b0+nb) for tap t.
                i, j = t // 7, t % 7
                if t == 24:
                    return h0b[:, b0 : b0 + nb, :]
                dr, dc = 3 - i, 3 - j
                if j % 2 == 0:
                    return s0[:, dr + 3 + b0 : dr + 3 + b0 + nb, 3 + dc : 515 + dc]
                return s1[:, dr + 3 + b0 : dr + 3 + b0 + nb, 4 + dc : 516 + dc]

            # Taps 0..43 stream full-tile; weight DMAs all on the SP ring.
            for t in range(44):
                wf = wfp.tile([128, 4, 512], F32, tag="wf")
                nc.sync.dma_start(out=wf[:], in_=gw_r[t])
                wb = wbp.tile([128, 4, 512], BF16, tag="wb")
                nc.scalar.copy(out=wb[:], in_=wf[:])
                if t == 0:
                    nc.vector.tensor_tensor(
                        out=acc[:], in0=wb[:], in1=src_for(t), op=MULT
                    )
                else:
                    prod = prp.tile([128, 4, 512], BF16, tag="prod")
                    nc.vector.tensor_tensor(
                        out=prod[:], in0=wb[:], in1=src_for(t), op=MULT
                    )
                    nc.vector.tensor_tensor(
                        out=acc[:], in0=acc[:], in1=prod[:], op=ADD
                    )

            # Tail: the cast->mult->add latency after the LAST weight bytes is
            # what the end of the kernel waits on, so shrink the pieces as the
            # stream drains: taps 44/45 run as half-tiles, 46/47 as per-block
            # quarters (bf16 cast path), and tap 48's quarters multiply
            # straight from the f32 weights (no cast hop) with the final add
            # emitting f32 into outf. Stores issue from SP after the quarters.
            for t in (44, 45):
                for h in (0, 1):
                    wfh = whp.tile([128, 2, 512], F32, tag="wfh")
                    nc.sync.dma_start(out=wfh[:], in_=gw_r[t][:, 2 * h : 2 * h + 2, :])
                    wbh = wsp.tile([128, 2, 512], BF16, tag="wbh")
                    nc.scalar.copy(out=wbh[:], in_=wfh[:])
                    prodh = prp.tile([128, 2, 512], BF16, tag="prodh")
                    nc.vector.tensor_tensor(
                        out=prodh[:], in0=wbh[:], in1=src_for(t, 2 * h, 2), op=MULT
                    )
                    nc.vector.tensor_tensor(
                        out=acc[:, 2 * h : 2 * h + 2, :],
                        in0=acc[:, 2 * h : 2 * h + 2, :],
                        in1=prodh[:],
                        op=ADD,
                    )
            for t in (46, 47):
                for b in range(4):
                    wq = wqp.tile([128, 512], F32, tag="wq")
                    nc.sync.dma_start(out=wq[:], in_=gw_r[t][:, b, :])
                    wbq = wsp.tile([128, 512], BF16, tag="wbq")
                    nc.scalar.copy(out=wbq[:], in_=wq[:])
                    prodq = pqp.tile([128, 512], BF16, tag="prodq")
                    nc.vector.tensor_tensor(
                        out=prodq[:], in0=wbq[:], in1=src_for(t, b, 1)[:, 0, :], op=MULT
                    )
                    nc.vector.tensor_tensor(
                        out=acc[:, b, :], in0=acc[:, b, :], in1=prodq[:], op=ADD
                    )
            for b in range(4):
                wq = wqp.tile([128, 512], F32, tag="wq")
                nc.sync.dma_start(out=wq[:], in_=gw_r[48][:, b, :])
                prodq = pqp.tile([128, 512], BF16, tag="prodq")
                nc.vector.tensor_tensor(
                    out=prodq[:], in0=wq[:], in1=src_for(48, b, 1)[:, 0, :], op=MULT
                )
                nc.vector.tensor_tensor(
                    out=outf[:, b, :], in0=acc[:, b, :], in1=prodq[:], op=ADD
                )
            for b in range(4):
                nc.sync.dma_start(out=out_r[:, b, :], in_=outf[:, b, :])

    nc.compile()
    return nc


def get_nc():
    if "nc" not in _CACHE:
        _CACHE["nc"] = _build_nc()
    return _CACHE["nc"]


def kernel(guide_weight, hn, h0):
    from concourse.bass_utils import run_bass_kernel_spmd

    nc = get_nc()
    in_maps = [
        {
            "gw": np.ascontiguousarray(guide_weight[b], dtype=np.float32),
            "hn": np.ascontiguousarray(hn[b, 0], dtype=np.float32),
            "h0": np.ascontiguousarray(h0[b, 0], dtype=np.float32),
        }
        for b in range(8)
    ]
    res = run_bass_kernel_spmd(nc, in_maps, core_ids=list(range(8)))
    return np.stack([res.results[b]["out"] for b in range(8)])[:, None].astype(
        np.float32
    )


# revision 10
# speedup vs baseline: 1.1316x; 1.0108x over previous
"""CSPN (7x7 per-pixel spatial propagation) Trainium2 kernel.

Problem: out[b,0,y,x] = sum_{i,j in 0..6} gw[b, 7i+j, y+3, x+3] * src(y+3-i, x+3-j)
where src = hn (zero-padded outside [0,512)) except the center tap (i=j=3)
which uses h0. Shapes: gw [8,49,518,518] f32, hn/h0 [8,1,512,512] f32.

Strategy: pure data parallel - one batch element per NeuronCore (8 cores).

Layout: image row r lives at partition r//4, row-block b = r%4 (each
partition holds 4 consecutive rows). A vertical shift of up to +-3 rows
then moves at most +-1 partition, and every shifted read window is a
plain strided slice of ONE halo tensor:

    s0[p, m, 3+c] = hn[4p + m - 3, c]   for m in [0, 10)

Tap (i, j) with dr = 3-i, dc = 3-j reads s0[:, dr+3 : dr+7, 3+dc : 515+dc].
The slice's element offset is even exactly when j is even; a second copy
s1 one slot to the right serves odd-j taps so bf16 DVE reads stay
4B-aligned (2x mode).

The halo is built with a single 1 MB DMA of hn: the mid blocks (m=3..6)
are cast from the f32 staging tile, and the +-1-partition-shifted blocks
(m=0..2, 7..9) are produced on the otherwise-idle PE array as matmuls
with sub/super-diagonal permutation matrices (built via affine_select),
whose all-zero edge columns also provide the top/bottom zero padding for
free. After startup the DMA engines stream nothing but the 49
guide-weight planes (51.4 MB/core - the memory-roofline term), h0, and
the final 1 MB output store; the modeled DMA bus never idles mid-stream.

Engine roles:
 - SP sequencer: pure DMA issue ring (staging, h0, every weight plane,
   output stores) - no compute waits can stall it.
 - ACT: all f32->bf16 weight casts (so DVE multiplies run in 2x mode).
 - DVE: per-tap multiply + accumulate chain (bf16).
 - GpSimd (Pool): takes six early taps on a second accumulator (merged
   once mid-stream) plus the first tail tap per block, keeping DVE
   slack so the final adds fire as soon as the last weight bytes land.
 - PE: the six halo shift matmuls.

Tap order runs the shift-free row (i=3) first and the i=0 row last, so
the PE-produced halo blocks are needed only well after they are ready.
Tail: the last taps shrink to half-tiles then per-row-block quarters;
the final tap's quarters multiply straight from f32 weights (no cast
hop) and the final add emits f32 into outf, which SP streams out.
"""

import numpy as np

_CACHE = {}

# Row i=3 (no vertical shift) first, i=0 (needs all up-blocks) last.
TAP_ORDER = [7 * i + j for i in (3, 4, 2, 5, 1, 6, 0) for j in range(7)]
POOL_POS = (1, 4, 7, 10, 13, 16)  # positions offloaded to GpSimd
MERGE_POS = 30  # position after which acc2 merges into acc


def _build_nc():
    import concourse.bacc as bacc
    import concourse.mybir as mybir
    import concourse.tile as tile

    F32 = mybir.dt.float32
    BF16 = mybir.dt.bfloat16
    MULT = mybir.AluOpType.mult
    ADD = mybir.AluOpType.add
    EQ = mybir.AluOpType.is_equal

    nc = bacc.Bacc("TRN2", target_bir_lowering=False, debug=False, num_devices=8)
    gw = nc.dram_tensor("gw", [49, 518, 518], F32, kind="ExternalInput").ap()
    hn = nc.dram_tensor("hn", [512, 512], F32, kind="ExternalInput").ap()
    h0 = nc.dram_tensor("h0", [512, 512], F32, kind="ExternalInput").ap()
    out = nc.dram_tensor("out", [512, 512], F32, kind="ExternalOutput").ap()

    with tile.TileContext(nc) as tc:
        with (
            tc.tile_pool(name="persist", bufs=1) as pp,
            tc.tile_pool(name="wf", bufs=4) as wfp,
            tc.tile_pool(name="wb", bufs=4) as wbp,
            tc.tile_pool(name="wb2", bufs=2) as wb2p,
            tc.tile_pool(name="prod", bufs=2) as prp,
            tc.tile_pool(name="p2", bufs=2) as p2p,
            tc.tile_pool(name="wq12", bufs=12) as wqp,
            tc.tile_pool(name="whalf", bufs=4) as whp,
            tc.tile_pool(name="wsmall", bufs=4) as wsp,
            tc.tile_pool(name="prodq", bufs=6) as pqp,
            tc.tile_pool(name="ps", bufs=2, space="PSUM") as psp,
        ):
            # --- halo staging -------------------------------------------
            stage = pp.tile([128, 4, 512], F32, tag="stage")
            nc.sync.dma_start(out=stage[:], in_=hn.rearrange("(p b) x -> p b x", b=4))
            h0f = pp.tile([128, 4, 512], F32)
            nc.sync.dma_start(out=h0f[:], in_=h0.rearrange("(p b) x -> p b x", b=4))

            s0 = pp.tile([128, 10, 520], BF16, tag="s0")
            s1 = pp.tile([128, 10, 520], BF16, tag="s1")
            nc.vector.memset(s0[:, :, 0:3], 0.0)
            nc.vector.memset(s0[:, :, 515:520], 0.0)
            nc.vector.memset(s1[:, :, 0:4], 0.0)
            nc.vector.memset(s1[:, :, 516:520], 0.0)
            nc.scalar.copy(out=s0[:, 3:7, 3:515], in_=stage[:])
            nc.vector.tensor_copy(s1[:, 3:7, 4:516], s0[:, 3:7, 3:515])
            h0b = pp.tile([128, 4, 512], BF16)
            nc.scalar.copy(out=h0b[:], in_=h0f[:])

            # Shift matrices: Tup[q, p] = [q == p+1], Tdn[q, p] = [q == p-1].
            # As matmul lhsT they realize out[p] = in[p+-1]; their all-zero
            # first/last columns zero the out-of-image rows automatically.
            ones = pp.tile([128, 128], BF16, tag="ones")
            nc.gpsimd.memset(ones[:], 1.0)
            tup = pp.tile([128, 128], BF16, tag="tup")
            nc.gpsimd.affine_select(
                out=tup[:], in_=ones[:], pattern=[[-1, 128]], compare_op=EQ,
                fill=0.0, base=-1, channel_multiplier=1,
            )
            tdn = pp.tile([128, 128], BF16, tag="tdn")
            nc.gpsimd.affine_select(
                out=tdn[:], in_=ones[:], pattern=[[-1, 128]], compare_op=EQ,
                fill=0.0, base=1, channel_multiplier=1,
            )

            # Up blocks m=7+r hold row 4p+4+r = mid block r of partition p+1;
            # dn blocks m=r hold row 4p+r-3 = mid block r+1 of partition p-1.
            # rhs reads the 4B-aligned s1 mid copy. Emission order matches
            # first use: i=4 needs m=2 first, i=2 needs m=7, etc.
            for kind, r in (("dn", 2), ("up", 0), ("dn", 1), ("up", 1), ("dn", 0), ("up", 2)):
                ps = psp.tile([128, 512], F32, tag="ps")
                if kind == "up":
                    nc.tensor.matmul(ps[:], tup[:], s1[:, 3 + r, 4:516])
                    dst = 7 + r
                else:
                    nc.tensor.matmul(ps[:], tdn[:], s1[:, 4 + r, 4:516])
                    dst = r
                nc.scalar.copy(out=s0[:, dst, 3:515], in_=ps[:])
            nc.vector.tensor_copy(s1[:, 0:3, 4:516], s0[:, 0:3, 3:515])
            nc.vector.tensor_copy(s1[:, 7:10, 4:516], s0[:, 7:10, 3:515])

            # --- tap machinery ------------------------------------------
            acc = pp.tile([128, 4, 512], BF16)
            acc2 = pp.tile([128, 4, 512], BF16)
            outf = pp.tile([128, 4, 512], F32)
            out_r = out.rearrange("(p b) x -> p b x", b=4)
            gw_r = [
                gw[t, 3:515, 3:515].rearrange("(p b) x -> p b x", b=4)
                for t in range(49)
            ]

            def src_for(t, b0=0, nb=4):
                i, j = t // 7, t % 7
                if t == 24:
                    return h0b[:, b0 : b0 + nb, :]
                dr, dc = 3 - i, 3 - j
                if j % 2 == 0:
                    return s0[:, dr + 3 + b0 : dr + 3 + b0 + nb, 3 + dc : 515 + dc]
                return s1[:, dr + 3 + b0 : dr + 3 + b0 + nb, 4 + dc : 516 + dc]

            # Positions 0..43 stream full-tile; six of them accumulate on
            # GpSimd into acc2, merged into acc once at MERGE_POS.
            pool_started = False
            for pos in range(44):
                t = TAP_ORDER[pos]
                wf = wfp.tile([128, 4, 512], F32, tag="wf")
                nc.sync.dma_start(out=wf[:], in_=gw_r[t])
                if pos in POOL_POS:
                    wb = wb2p.tile([128, 4, 512], BF16, tag="wb2")
                    nc.scalar.copy(out=wb[:], in_=wf[:])
                    if not pool_started:
                        nc.gpsimd.tensor_tensor(
                            out=acc2[:], in0=wb[:], in1=src_for(t), op=MULT
                        )
                        pool_started = True
                    else:
                        p2 = p2p.tile([128, 4, 512], BF16, tag="p2")
                        nc.gpsimd.tensor_tensor(
                            out=p2[:], in0=wb[:], in1=src_for(t), op=MULT
                        )
                        nc.gpsimd.tensor_tensor(
                            out=acc2[:], in0=acc2[:], in1=p2[:], op=ADD
                        )
                else:
                    wb = wbp.tile([128, 4, 512], BF16, tag="wb")
                    nc.scalar.copy(out=wb[:], in_=wf[:])
                    if pos == 0:
                        nc.vector.tensor_tensor(
                            out=acc[:], in0=wb[:], in1=src_for(t), op=MULT
                        )
                    else:
                        prod = prp.tile([128, 4, 512], BF16, tag="prod")
                        nc.vector.tensor_tensor(
                            out=prod[:], in0=wb[:], in1=src_for(t), op=MULT
                        )
                        nc.vector.tensor_tensor(
                            out=acc[:], in0=acc[:], in1=prod[:], op=ADD
                        )
                if pos == MERGE_POS:
                    nc.vector.tensor_tensor(out=acc[:], in0=acc[:], in1=acc2[:], op=ADD)

            # --- tail ----------------------------------------------------
            # Positions 44/45 as half-tiles on DVE; 46 per-block on GpSimd;
            # 47 per-block on DVE; 48 multiplies straight from f32 weights
            # with the final add emitting f32 into outf. Stores follow on SP.
            for pos in (44, 45):
                t = TAP_ORDER[pos]
                for h in (0, 1):
                    wfh = whp.tile([128, 2, 512], F32, tag="wfh")
                    nc.sync.dma_start(out=wfh[:], in_=gw_r[t][:, 2 * h : 2 * h + 2, :])
                    wbh = wsp.tile([128, 2, 512], BF16, tag="wbh")
                    nc.scalar.copy(out=wbh[:], in_=wfh[:])
                    prodh = prp.tile([128, 2, 512], BF16, tag="prodh")
                    nc.vector.tensor_tensor(
                        out=prodh[:], in0=wbh[:], in1=src_for(t, 2 * h, 2), op=MULT
                    )
                    nc.vector.tensor_tensor(
                        out=acc[:, 2 * h : 2 * h + 2, :],
                        in0=acc[:, 2 * h : 2 * h + 2, :],
                        in1=prodh[:],
                        op=ADD,
                    )
            tA, tB, tC = TAP_ORDER[46], TAP_ORDER[47], TAP_ORDER[48]
            for b in range(4):
                wq = wqp.tile([128, 512], F32, tag="wq")
                nc.sync.dma_start(out=wq[:], in_=gw_r[tA][:, b, :])
                wbq = wsp.tile([128, 512], BF16, tag="wbq")
                nc.scalar.copy(out=wbq[:], in_=wq[:])
                pq = pqp.tile([128, 512], BF16, tag="pq46")
                nc.gpsimd.tensor_tensor(
                    out=pq[:], in0=wbq[:], in1=src_for(tA, b, 1)[:, 0, :], op=MULT
                )
                nc.gpsimd.tensor_tensor(
                    out=acc[:, b, :], in0=acc[:, b, :], in1=pq[:], op=ADD
                )
            for b in range(4):
                wq = wqp.tile([128, 512], F32, tag="wq")
                nc.sync.dma_start(out=wq[:], in_=gw_r[tB][:, b, :])
                wbq = wsp.tile([128, 512], BF16, tag="wbq")
                nc.scalar.copy(out=wbq[:], in_=wq[:])
                pq = pqp.tile([128, 512], BF16, tag="prodq")
                nc.vector.tensor_tensor(
                    out=pq[:], in0=wbq[:], in1=src_for(tB, b, 1)[:, 0, :], op=MULT
                )
                nc.vector.tensor_tensor(
                    out=acc[:, b, :], in0=acc[:, b, :], in1=pq[:], op=ADD
                )
            for b in range(4):
                wq = wqp.tile([128, 512], F32, tag="wq")
                nc.sync.dma_start(out=wq[:], in_=gw_r[tC][:, b, :])
                pq = pqp.tile([128, 512], BF16, tag="prodq")
                nc.vector.tensor_tensor(
                    out=pq[:], in0=wq[:], in1=src_for(tC, b, 1)[:, 0, :], op=MULT
                )
                nc.vector.tensor_tensor(
                    out=outf[:, b, :], in0=acc[:, b, :], in1=pq[:], op=ADD
                )
            for b in range(4):
                nc.sync.dma_start(out=out_r[:, b, :], in_=outf[:, b, :])

    nc.compile()
    return nc


def get_nc():
    if "nc" not in _CACHE:
        _CACHE["nc"] = _build_nc()
    return _CACHE["nc"]


def kernel(guide_weight, hn, h0):
    from concourse.bass_utils import run_bass_kernel_spmd

    nc = get_nc()
    in_maps = [
        {
            "gw": np.ascontiguousarray(guide_weight[b], dtype=np.float32),
            "hn": np.ascontiguousarray(hn[b, 0], dtype=np.float32),
            "h0": np.ascontiguousarray(h0[b, 0], dtype=np.float32),
        }
        for b in range(8)
    ]
    res = run_bass_kernel_spmd(nc, in_maps, core_ids=list(range(8)))
    return np.stack([res.results[b]["out"] for b in range(8)])[:, None].astype(
        np.float32
    )


# revision 11
# speedup vs baseline: 1.1575x; 1.0229x over previous
"""CSPN (7x7 per-pixel spatial propagation) Trainium2 kernel.

Problem: out[b,0,y,x] = sum_{i,j in 0..6} gw[b, 7i+j, y+3, x+3] * src(y+3-i, x+3-j)
where src = hn (zero-padded outside [0,512)) except the center tap (i=j=3)
which uses h0. Shapes: gw [8,49,518,518] f32, hn/h0 [8,1,512,512] f32.

Strategy: pure data parallel - one batch element per NeuronCore (8 cores).

Layout: image row r lives at partition r//4, row-block b = r%4 (each
partition holds 4 consecutive rows). A vertical shift of up to +-3 rows
then moves at most +-1 partition, and every shifted read window is a
plain strided slice of ONE halo tensor:

    s0[p, m, 3+c] = hn[4p + m - 3, c]   for m in [0, 10)

Tap (i, j) with dr = 3-i, dc = 3-j reads s0[:, dr+3 : dr+7, 3+dc : 515+dc].
The slice's element offset is even exactly when j is even; a second copy
s1 one slot to the right serves odd-j taps so bf16 DVE reads stay
4B-aligned (2x mode).

The halo is built with a single 1 MB DMA of hn: the mid blocks (m=3..6)
are cast from the f32 staging tile, and the +-1-partition-shifted blocks
(m=0..2, 7..9) are produced on the otherwise-idle PE array as matmuls
with sub/super-diagonal permutation matrices (built via affine_select),
whose all-zero edge columns also provide the top/bottom zero padding for
free. After startup the DMA engines stream nothing but the 49
guide-weight planes (51.4 MB/core - the memory-roofline term), h0, and
the final 1 MB output store; the modeled DMA bus never idles mid-stream.

Engine roles:
 - SP sequencer: pure DMA issue ring (staging, h0, every weight plane,
   output stores) - no compute waits can stall it.
 - ACT: all f32->bf16 weight casts (so DVE multiplies run in 2x mode).
 - DVE: per-tap multiply + accumulate chain (bf16).
 - GpSimd (Pool): takes six early taps on a second accumulator (merged
   once mid-stream) plus the first tail tap per block, keeping DVE
   slack so the final adds fire as soon as the last weight bytes land.
 - PE: the six halo shift matmuls.

Tap order runs the shift-free row (i=3) first and the i=0 row last, so
the PE-produced halo blocks are needed only well after they are ready.
Tail: the last taps shrink to half-tiles then per-row-block quarters;
the final tap's quarters multiply straight from f32 weights (no cast
hop) and the final add emits f32 into outf, which SP streams out.
"""

import numpy as np

_CACHE = {}

# Row i=3 (no vertical shift) first, i=0 (needs all up-blocks) last.
TAP_ORDER = [7 * i + j for i in (3, 4, 2, 5, 1, 6, 0) for j in range(7)]
POOL_POS = (1, 4, 7, 10, 13, 16)  # positions offloaded to GpSimd
MERGE_POS = 30  # position after which acc2 merges into acc


def _build_nc():
    import concourse.bacc as bacc
    import concourse.mybir as mybir
    import concourse.tile as tile

    F32 = mybir.dt.float32
    BF16 = mybir.dt.bfloat16
    MULT = mybir.AluOpType.mult
    ADD = mybir.AluOpType.add
    EQ = mybir.AluOpType.is_equal

    nc = bacc.Bacc("TRN2", target_bir_lowering=False, debug=False, num_devices=8)
    gw = nc.dram_tensor("gw", [49, 518, 518], F32, kind="ExternalInput").ap()
    hn = nc.dram_tensor("hn", [512, 512], F32, kind="ExternalInput").ap()
    h0 = nc.dram_tensor("h0", [512, 512], F32, kind="ExternalInput").ap()
    out = nc.dram_tensor("out", [512, 512], F32, kind="ExternalOutput").ap()

    with tile.TileContext(nc) as tc:
        with (
            tc.tile_pool(name="persist", bufs=1) as pp,
            tc.tile_pool(name="wf", bufs=4) as wfp,
            tc.tile_pool(name="wb", bufs=4) as wbp,
            tc.tile_pool(name="wb2", bufs=2) as wb2p,
            tc.tile_pool(name="prod", bufs=2) as prp,
            tc.tile_pool(name="p2", bufs=2) as p2p,
            tc.tile_pool(name="wq12", bufs=12) as wqp,
            tc.tile_pool(name="whalf", bufs=4) as whp,
            tc.tile_pool(name="wsmall", bufs=6) as wsp,
            tc.tile_pool(name="prodq", bufs=6) as pqp,
            tc.tile_pool(name="ps", bufs=2, space="PSUM") as psp,
        ):
            # --- halo staging -------------------------------------------
            stage = pp.tile([128, 4, 512], F32, tag="stage")
            nc.sync.dma_start(out=stage[:], in_=hn.rearrange("(p b) x -> p b x", b=4))
            h0f = pp.tile([128, 4, 512], F32)
            nc.sync.dma_start(out=h0f[:], in_=h0.rearrange("(p b) x -> p b x", b=4))

            s0 = pp.tile([128, 10, 520], BF16, tag="s0")
            s1 = pp.tile([128, 10, 520], BF16, tag="s1")
            nc.vector.memset(s0[:, :, 0:3], 0.0)
            nc.vector.memset(s0[:, :, 515:520], 0.0)
            nc.vector.memset(s1[:, :, 0:4], 0.0)
            nc.vector.memset(s1[:, :, 516:520], 0.0)
            nc.scalar.copy(out=s0[:, 3:7, 3:515], in_=stage[:])
            nc.vector.tensor_copy(s1[:, 3:7, 4:516], s0[:, 3:7, 3:515])
            h0b = pp.tile([128, 4, 512], BF16)
            nc.scalar.copy(out=h0b[:], in_=h0f[:])

            # Shift matrices: Tup[q, p] = [q == p+1], Tdn[q, p] = [q == p-1].
            # As matmul lhsT they realize out[p] = in[p+-1]; their all-zero
            # first/last columns zero the out-of-image rows automatically.
            ones = pp.tile([128, 128], BF16, tag="ones")
            nc.gpsimd.memset(ones[:], 1.0)
            tup = pp.tile([128, 128], BF16, tag="tup")
            nc.gpsimd.affine_select(
                out=tup[:], in_=ones[:], pattern=[[-1, 128]], compare_op=EQ,
                fill=0.0, base=-1, channel_multiplier=1,
            )
            tdn = pp.tile([128, 128], BF16, tag="tdn")
            nc.gpsimd.affine_select(
                out=tdn[:], in_=ones[:], pattern=[[-1, 128]], compare_op=EQ,
                fill=0.0, base=1, channel_multiplier=1,
            )

            # Up blocks m=7+r hold row 4p+4+r = mid block r of partition p+1;
            # dn blocks m=r hold row 4p+r-3 = mid block r+1 of partition p-1.
            # rhs reads the 4B-aligned s1 mid copy. Emission order matches
            # first use: i=4 needs m=2 first, i=2 needs m=7, etc.
            for kind, r in (("dn", 2), ("up", 0), ("dn", 1), ("up", 1), ("dn", 0), ("up", 2)):
                ps = psp.tile([128, 512], F32, tag="ps")
                if kind == "up":
                    nc.tensor.matmul(ps[:], tup[:], s1[:, 3 + r, 4:516])
                    dst = 7 + r
                else:
                    nc.tensor.matmul(ps[:], tdn[:], s1[:, 4 + r, 4:516])
                    dst = r
                nc.scalar.copy(out=s0[:, dst, 3:515], in_=ps[:])
            nc.vector.tensor_copy(s1[:, 0:3, 4:516], s0[:, 0:3, 3:515])
            nc.vector.tensor_copy(s1[:, 7:10, 4:516], s0[:, 7:10, 3:515])

            # --- tap machinery ------------------------------------------
            acc = pp.tile([128, 4, 512], BF16)
            acc2 = pp.tile([128, 4, 512], BF16)
            outf = pp.tile([128, 4, 512], F32)
            out_r = out.rearrange("(p b) x -> p b x", b=4)
            gw_r = [
                gw[t, 3:515, 3:515].rearrange("(p b) x -> p b x", b=4)
                for t in range(49)
            ]

            def src_for(t, b0=0, nb=4):
                i, j = t // 7, t % 7
                if t == 24:
                    return h0b[:, b0 : b0 + nb, :]
                dr, dc = 3 - i, 3 - j
                if j % 2 == 0:
                    return s0[:, dr + 3 + b0 : dr + 3 + b0 + nb, 3 + dc : 515 + dc]
                return s1[:, dr + 3 + b0 : dr + 3 + b0 + nb, 4 + dc : 516 + dc]

            # Positions 0..43 stream full-tile; six of them accumulate on
            # GpSimd into acc2, merged into acc once at MERGE_POS.
            pool_started = False
            for pos in range(40):
                t = TAP_ORDER[pos]
                wf = wfp.tile([128, 4, 512], F32, tag="wf")
                nc.sync.dma_start(out=wf[:], in_=gw_r[t])
                if pos in POOL_POS:
                    wb = wb2p.tile([128, 4, 512], BF16, tag="wb2")
                    nc.scalar.copy(out=wb[:], in_=wf[:])
                    if not pool_started:
                        nc.gpsimd.tensor_tensor(
                            out=acc2[:], in0=wb[:], in1=src_for(t), op=MULT
                        )
                        pool_started = True
                    else:
                        p2 = p2p.tile([128, 4, 512], BF16, tag="p2")
                        nc.gpsimd.tensor_tensor(
                            out=p2[:], in0=wb[:], in1=src_for(t), op=MULT
                        )
                        nc.gpsimd.tensor_tensor(
                            out=acc2[:], in0=acc2[:], in1=p2[:], op=ADD
                        )
                else:
                    wb = wbp.tile([128, 4, 512], BF16, tag="wb")
                    nc.scalar.copy(out=wb[:], in_=wf[:])
                    if pos == 0:
                        nc.vector.tensor_tensor(
                            out=acc[:], in0=wb[:], in1=src_for(t), op=MULT
                        )
                    else:
                        prod = prp.tile([128, 4, 512], BF16, tag="prod")
                        nc.vector.tensor_tensor(
                            out=prod[:], in0=wb[:], in1=src_for(t), op=MULT
                        )
                        nc.vector.tensor_tensor(
                            out=acc[:], in0=acc[:], in1=prod[:], op=ADD
                        )
                if pos == MERGE_POS:
                    nc.vector.tensor_tensor(out=acc[:], in0=acc[:], in1=acc2[:], op=ADD)

            # --- tail ----------------------------------------------------
            # The kernel's end waits on the cast->mult->add chain behind the
            # LAST weight bytes, so taper the pieces as the stream drains:
            # positions 40..43 run as half-tiles and 44..48 as per-row-block
            # quarters. The final tap's add emits f32 straight into outf
            # (no output cast), and SP streams the four block stores out.
            for pos in (40, 41, 42, 43):
                t = TAP_ORDER[pos]
                for h in (0, 1):
                    wfh = whp.tile([128, 2, 512], F32, tag="wfh")
                    nc.sync.dma_start(out=wfh[:], in_=gw_r[t][:, 2 * h : 2 * h + 2, :])
                    wbh = wsp.tile([128, 2, 512], BF16, tag="wbh")
                    nc.scalar.copy(out=wbh[:], in_=wfh[:])
                    prodh = prp.tile([128, 2, 512], BF16, tag="prodh")
                    nc.vector.tensor_tensor(
                        out=prodh[:], in0=wbh[:], in1=src_for(t, 2 * h, 2), op=MULT
                    )
                    nc.vector.tensor_tensor(
                        out=acc[:, 2 * h : 2 * h + 2, :],
                        in0=acc[:, 2 * h : 2 * h + 2, :],
                        in1=prodh[:],
                        op=ADD,
                    )
            for pos in (44, 45, 46, 47, 48):
                t = TAP_ORDER[pos]
                last = pos == 48
                for b in range(4):
                    wq = wqp.tile([128, 512], F32, tag="wq")
                    nc.sync.dma_start(out=wq[:], in_=gw_r[t][:, b, :])
                    wbq = wsp.tile([128, 512], BF16, tag="wbq")
                    nc.scalar.copy(out=wbq[:], in_=wq[:])
                    pq = pqp.tile([128, 512], BF16, tag="prodq")
                    nc.vector.tensor_tensor(
                        out=pq[:], in0=wbq[:], in1=src_for(t, b, 1)[:, 0, :], op=MULT
                    )
                    if last:
                        nc.vector.tensor_tensor(
                            out=outf[:, b, :], in0=acc[:, b, :], in1=pq[:], op=ADD
                        )
                    else:
                        nc.vector.tensor_tensor(
                            out=acc[:, b, :], in0=acc[:, b, :], in1=pq[:], op=ADD
                        )
            for b in range(4):
                nc.sync.dma_start(out=out_r[:, b, :], in_=outf[:, b, :])

    nc.compile()
    return nc


def get_nc():
    if "nc" not in _CACHE:
        _CACHE["nc"] = _build_nc()
    return _CACHE["nc"]


def kernel(guide_weight, hn, h0):
    from concourse.bass_utils import run_bass_kernel_spmd

    nc = get_nc()
    in_maps = [
        {
            "gw": np.ascontiguousarray(guide_weight[b], dtype=np.float32),
            "hn": np.ascontiguousarray(hn[b, 0], dtype=np.float32),
            "h0": np.ascontiguousarray(h0[b, 0], dtype=np.float32),
        }
        for b in range(8)
    ]
    res = run_bass_kernel_spmd(nc, in_maps, core_ids=list(range(8)))
    return np.stack([res.results[b]["out"] for b in range(8)])[:, None].astype(
        np.float32
    )


# revision 12
# speedup vs baseline: 1.1580x; 1.0004x over previous
"""CSPN (7x7 per-pixel spatial propagation) Trainium2 kernel.

Problem: out[b,0,y,x] = sum_{i,j in 0..6} gw[b, 7i+j, y+3, x+3] * src(y+3-i, x+3-j)
where src = hn (zero-padded outside [0,512)) except the center tap (i=j=3)
which uses h0. Shapes: gw [8,49,518,518] f32, hn/h0 [8,1,512,512] f32.

Strategy: pure data parallel - one batch element per NeuronCore (8 cores).

Layout: image row r lives at partition r//4, row-block b = r%4 (each
partition holds 4 consecutive rows). A vertical shift of up to +-3 rows
then moves at most +-1 partition, and every shifted read window is a
plain strided slice of ONE halo tensor:

    s0[p, m, 3+c] = hn[4p + m - 3, c]   for m in [0, 10)

Tap (i, j) with dr = 3-i, dc = 3-j reads s0[:, dr+3 : dr+7, 3+dc : 515+dc].
The slice's element offset is even exactly when j is even; a second copy
s1 one slot to the right serves odd-j taps so bf16 DVE reads stay
4B-aligned (2x mode).

The halo is built with a single 1 MB DMA of hn: the mid blocks (m=3..6)
are cast from the f32 staging tile, and the +-1-partition-shifted blocks
(m=0..2, 7..9) are produced on the otherwise-idle PE array as matmuls
with sub/super-diagonal permutation matrices (built via affine_select),
whose all-zero edge columns also provide the top/bottom zero padding for
free. After startup the DMA engines stream nothing but the 49
guide-weight planes (51.4 MB/core - the memory-roofline term), h0, and
the final 1 MB output store; the modeled DMA bus never idles mid-stream.

Engine roles:
 - SP sequencer: pure DMA issue ring (staging, h0, every weight plane,
   output stores) - no compute waits can stall it.
 - ACT: all f32->bf16 weight casts (so DVE multiplies run in 2x mode).
 - DVE: per-tap multiply + accumulate chain (bf16).
 - GpSimd (Pool): takes six early taps on a second accumulator (merged
   once mid-stream) plus the first tail tap per block, keeping DVE
   slack so the final adds fire as soon as the last weight bytes land.
 - PE: the six halo shift matmuls.

Tap order runs the shift-free row (i=3) first and the i=0 row last, so
the PE-produced halo blocks are needed only well after they are ready.
Tail: the last taps shrink to half-tiles then per-row-block quarters;
the final tap's quarters multiply straight from f32 weights (no cast
hop) and the final add emits f32 into outf, which SP streams out.
"""

import numpy as np

_CACHE = {}

# Row i=3 (no vertical shift) first, i=0 (needs all up-blocks) last.
TAP_ORDER = [7 * i + j for i in (3, 4, 2, 5, 1, 6, 0) for j in range(7)]
POOL_POS = (1, 4, 7, 10, 13, 16)  # positions offloaded to GpSimd
MERGE_POS = 30  # position after which acc2 merges into acc


def _build_nc():
    import concourse.bacc as bacc
    import concourse.mybir as mybir
    import concourse.tile as tile

    F32 = mybir.dt.float32
    BF16 = mybir.dt.bfloat16
    MULT = mybir.AluOpType.mult
    ADD = mybir.AluOpType.add
    EQ = mybir.AluOpType.is_equal

    nc = bacc.Bacc("TRN2", target_bir_lowering=False, debug=False, num_devices=8)
    gw = nc.dram_tensor("gw", [49, 518, 518], F32, kind="ExternalInput").ap()
    hn = nc.dram_tensor("hn", [512, 512], F32, kind="ExternalInput").ap()
    h0 = nc.dram_tensor("h0", [512, 512], F32, kind="ExternalInput").ap()
    out = nc.dram_tensor("out", [512, 512], F32, kind="ExternalOutput").ap()

    with tile.TileContext(nc) as tc:
        with (
            tc.tile_pool(name="persist", bufs=1) as pp,
            tc.tile_pool(name="wf", bufs=4) as wfp,
            tc.tile_pool(name="wb", bufs=4) as wbp,
            tc.tile_pool(name="wb2", bufs=2) as wb2p,
            tc.tile_pool(name="prod", bufs=2) as prp,
            tc.tile_pool(name="p2", bufs=2) as p2p,
            tc.tile_pool(name="wq12", bufs=12) as wqp,
            tc.tile_pool(name="whalf", bufs=4) as whp,
            tc.tile_pool(name="wsmall", bufs=6) as wsp,
            tc.tile_pool(name="prodq", bufs=6) as pqp,
            tc.tile_pool(name="ps", bufs=2, space="PSUM") as psp,
        ):
            # --- halo staging -------------------------------------------
            stage = pp.tile([128, 4, 512], F32, tag="stage")
            nc.sync.dma_start(out=stage[:], in_=hn.rearrange("(p b) x -> p b x", b=4))
            h0f = pp.tile([128, 4, 512], F32)
            nc.sync.dma_start(out=h0f[:], in_=h0.rearrange("(p b) x -> p b x", b=4))

            s0 = pp.tile([128, 10, 520], BF16, tag="s0")
            s1 = pp.tile([128, 10, 520], BF16, tag="s1")
            nc.vector.memset(s0[:, :, 0:3], 0.0)
            nc.vector.memset(s0[:, :, 515:520], 0.0)
            nc.vector.memset(s1[:, :, 0:4], 0.0)
            nc.vector.memset(s1[:, :, 516:520], 0.0)
            nc.scalar.copy(out=s0[:, 3:7, 3:515], in_=stage[:])
            nc.vector.tensor_copy(s1[:, 3:7, 4:516], s0[:, 3:7, 3:515])
            h0b = pp.tile([128, 4, 512], BF16)
            nc.scalar.copy(out=h0b[:], in_=h0f[:])

            # Shift matrices: Tup[q, p] = [q == p+1], Tdn[q, p] = [q == p-1].
            # As matmul lhsT they realize out[p] = in[p+-1]; their all-zero
            # first/last columns zero the out-of-image rows automatically.
            ones = pp.tile([128, 128], BF16, tag="ones")
            nc.gpsimd.memset(ones[:], 1.0)
            tup = pp.tile([128, 128], BF16, tag="tup")
            nc.gpsimd.affine_select(
                out=tup[:], in_=ones[:], pattern=[[-1, 128]], compare_op=EQ,
                fill=0.0, base=-1, channel_multiplier=1,
            )
            tdn = pp.tile([128, 128], BF16, tag="tdn")
            nc.gpsimd.affine_select(
                out=tdn[:], in_=ones[:], pattern=[[-1, 128]], compare_op=EQ,
                fill=0.0, base=1, channel_multiplier=1,
            )

            # Up blocks m=7+r hold row 4p+4+r = mid block r of partition p+1;
            # dn blocks m=r hold row 4p+r-3 = mid block r+1 of partition p-1.
            # rhs reads the 4B-aligned s1 mid copy. Emission order matches
            # first use: i=4 needs m=2 first, i=2 needs m=7, etc.
            for kind, r in (("dn", 2), ("up", 0), ("dn", 1), ("up", 1), ("dn", 0), ("up", 2)):
                ps = psp.tile([128, 512], F32, tag="ps")
                if kind == "up":
                    nc.tensor.matmul(ps[:], tup[:], s1[:, 3 + r, 4:516])
                    dst = 7 + r
                else:
                    nc.tensor.matmul(ps[:], tdn[:], s1[:, 4 + r, 4:516])
                    dst = r
                nc.scalar.copy(out=s0[:, dst, 3:515], in_=ps[:])
            nc.vector.tensor_copy(s1[:, 0:3, 4:516], s0[:, 0:3, 3:515])
            nc.vector.tensor_copy(s1[:, 7:10, 4:516], s0[:, 7:10, 3:515])

            # --- tap machinery ------------------------------------------
            acc = pp.tile([128, 4, 512], BF16)
            acc2 = pp.tile([128, 4, 512], BF16)
            outf = pp.tile([128, 4, 512], F32)
            out_r = out.rearrange("(p b) x -> p b x", b=4)
            gw_r = [
                gw[t, 3:515, 3:515].rearrange("(p b) x -> p b x", b=4)
                for t in range(49)
            ]

            def src_for(t, b0=0, nb=4):
                i, j = t // 7, t % 7
                if t == 24:
                    return h0b[:, b0 : b0 + nb, :]
                dr, dc = 3 - i, 3 - j
                if j % 2 == 0:
                    return s0[:, dr + 3 + b0 : dr + 3 + b0 + nb, 3 + dc : 515 + dc]
                return s1[:, dr + 3 + b0 : dr + 3 + b0 + nb, 4 + dc : 516 + dc]

            # Positions 0..43 stream full-tile; six of them accumulate on
            # GpSimd into acc2, merged into acc once at MERGE_POS.
            pool_started = False
            for pos in range(36):
                t = TAP_ORDER[pos]
                wf = wfp.tile([128, 4, 512], F32, tag="wf")
                nc.sync.dma_start(out=wf[:], in_=gw_r[t])
                if pos in POOL_POS:
                    wb = wb2p.tile([128, 4, 512], BF16, tag="wb2")
                    nc.scalar.copy(out=wb[:], in_=wf[:])
                    if not pool_started:
                        nc.gpsimd.tensor_tensor(
                            out=acc2[:], in0=wb[:], in1=src_for(t), op=MULT
                        )
                        pool_started = True
                    else:
                        p2 = p2p.tile([128, 4, 512], BF16, tag="p2")
                        nc.gpsimd.tensor_tensor(
                            out=p2[:], in0=wb[:], in1=src_for(t), op=MULT
                        )
                        nc.gpsimd.tensor_tensor(
                            out=acc2[:], in0=acc2[:], in1=p2[:], op=ADD
                        )
                else:
                    wb = wbp.tile([128, 4, 512], BF16, tag="wb")
                    nc.scalar.copy(out=wb[:], in_=wf[:])
                    if pos == 0:
                        nc.vector.tensor_tensor(
                            out=acc[:], in0=wb[:], in1=src_for(t), op=MULT
                        )
                    else:
                        prod = prp.tile([128, 4, 512], BF16, tag="prod")
                        nc.vector.tensor_tensor(
                            out=prod[:], in0=wb[:], in1=src_for(t), op=MULT
                        )
                        nc.vector.tensor_tensor(
                            out=acc[:], in0=acc[:], in1=prod[:], op=ADD
                        )
                if pos == MERGE_POS:
                    nc.vector.tensor_tensor(out=acc[:], in0=acc[:], in1=acc2[:], op=ADD)

            # --- tail ----------------------------------------------------
            # The kernel's end waits on the cast->mult->add chain behind the
            # LAST weight bytes, so taper the pieces as the stream drains:
            # positions 36..41 run as half-tiles and 42..48 as per-row-block
            # quarters. The final tap's add emits f32 straight into outf
            # (no output cast), and SP streams the four block stores out.
            for pos in (36, 37, 38, 39, 40, 41):
                t = TAP_ORDER[pos]
                for h in (0, 1):
                    wfh = whp.tile([128, 2, 512], F32, tag="wfh")
                    nc.sync.dma_start(out=wfh[:], in_=gw_r[t][:, 2 * h : 2 * h + 2, :])
                    wbh = wsp.tile([128, 2, 512], BF16, tag="wbh")
                    nc.scalar.copy(out=wbh[:], in_=wfh[:])
                    prodh = prp.tile([128, 2, 512], BF16, tag="prodh")
                    nc.vector.tensor_tensor(
                        out=prodh[:], in0=wbh[:], in1=src_for(t, 2 * h, 2), op=MULT
                    )
                    nc.vector.tensor_tensor(
                        out=acc[:, 2 * h : 2 * h + 2, :],
                        in0=acc[:, 2 * h : 2 * h + 2, :],
                        in1=prodh[:],
                        op=ADD,
                    )
            for pos in (42, 43, 44, 45, 46, 47, 48):
                t = TAP_ORDER[pos]
                last = pos == 48
                for b in range(4):
                    wq = wqp.tile([128, 512], F32, tag="wq")
                    nc.sync.dma_start(out=wq[:], in_=gw_r[t][:, b, :])
                    wbq = wsp.tile([128, 512], BF16, tag="wbq")
                    nc.scalar.copy(out=wbq[:], in_=wq[:])
                    pq = pqp.tile([128, 512], BF16, tag="prodq")
                    nc.vector.tensor_tensor(
                        out=pq[:], in0=wbq[:], in1=src_for(t, b, 1)[:, 0, :], op=MULT
                    )
                    if last:
                        nc.vector.tensor_tensor(
                            out=outf[:, b, :], in0=acc[:, b, :], in1=pq[:], op=ADD
                        )
                    else:
                        nc.vector.tensor_tensor(
                            out=acc[:, b, :], in0=acc[:, b, :], in1=pq[:], op=ADD
                        )
            for b in range(4):
                nc.sync.dma_start(out=out_r[:, b, :], in_=outf[:, b, :])

    nc.compile()
    return nc


def get_nc():
    if "nc" not in _CACHE:
        _CACHE["nc"] = _build_nc()
    return _CACHE["nc"]


def kernel(guide_weight, hn, h0):
    from concourse.bass_utils import run_bass_kernel_spmd

    nc = get_nc()
    in_maps = [
        {
            "gw": np.ascontiguousarray(guide_weight[b], dtype=np.float32),
            "hn": np.ascontiguousarray(hn[b, 0], dtype=np.float32),
            "h0": np.ascontiguousarray(h0[b, 0], dtype=np.float32),
        }
        for b in range(8)
    ]
    res = run_bass_kernel_spmd(nc, in_maps, core_ids=list(range(8)))
    return np.stack([res.results[b]["out"] for b in range(8)])[:, None].astype(
        np.float32
    )
